# revision 12
# baseline (speedup 1.0000x reference)
"""Diagonal SSM kernel for Trainium2 (8 NeuronCores, batch-parallel).

Computes, for x [8, 4096, 1024], W_decay/W_input [1024, 1024], biases [1024]:
    decays     = sigmoid(x @ W_decay.T + b_decay)
    injections = x @ W_input.T + b_input
    states_t   = decays_t * states_{t-1} + injections_t      (scan over T)

Sharding: batch b -> core b (8 batches, 8 cores, no collectives).

Per-core pipeline over 8 time-panels of 512:
  - x panel loaded with fp32->bf16 cast (SWDGE), PE-transposed to put the
    contraction dim d on partitions,
  - both projections as PE bf16 matmuls accumulating fp32 in PSUM,
  - sigmoid(z + b_decay) and (z + b_input) on the scalar engine straight
    out of PSUM,
  - the recurrence itself is a single native DVE tensor_tensor_scan per
    [128 channels x 512 steps] tile (fp32 state), chained across panels
    through its `initial` operand,
  - states PE-transposed back to [t, d] and stored fp32.

Measured ~230-430 us wall on HW (noise-limited measurement; cost model
predicts 307 us); PE-bound: 1024 bf16 matmuls ~218 us + 640 transposes.
bf16 projections give rel err ~1.9e-3 vs the fp32 reference (fp8 was
measured at 1.4e-2+ in numpy and rejected).
"""

import sys

if "/opt/trn_rl_repo" not in sys.path:
    sys.path.insert(0, "/opt/trn_rl_repo")

from contextlib import ExitStack

import numpy as np

import concourse.bass as bass  # noqa: F401  (engine types referenced via nc)
import concourse.tile as tile
from concourse import bacc, masks, mybir
from concourse.bass_utils import run_bass_kernel_spmd

N_CORES = 8
B, T, D, P = 8, 4096, 1024, 128
PANEL = 512                  # time-panel width (one PSUM bank of fp32)
N_PANELS = T // PANEL        # 8
TK = PANEL // P              # 4 row-blocks of 128 timesteps per panel
EB = D // P                  # 8 output-channel blocks
DB = D // P                  # 8 contraction blocks

F32 = mybir.dt.float32
BF16 = mybir.dt.bfloat16

_cached_nc = {}

# pool buffer depths (tuned via TimelineSim + HW checks)
CFG = {"xbf": 2, "xt": 2, "dec": 4, "st": 2, "ysb": 4}
# how x gets transposed to [d, t]: "pe" (tensor engine) or "dma" (xbar via
# a bf16 DRAM staging copy)
XT_MODE = "pe"


def _build(repeat: int = 1, ablate: frozenset = frozenset()):
    """Build the per-core program. `repeat` re-runs the panel pipeline
    (timing aid: slope between repeats isolates steady-state exec time)."""
    key = (repeat, ablate)
    if key in _cached_nc:
        return _cached_nc[key]

    nc = bacc.Bacc(
        "TRN2",
        target_bir_lowering=False,
        debug=False,
        enable_asserts=True,
        num_devices=N_CORES,
    )

    x_ap = nc.dram_tensor("x", [T, D], F32, kind="ExternalInput").ap()
    wd_ap = nc.dram_tensor("wd", [D, D], F32, kind="ExternalInput").ap()
    bd_ap = nc.dram_tensor("bd", [D], F32, kind="ExternalInput").ap()
    wi_ap = nc.dram_tensor("wi", [D, D], F32, kind="ExternalInput").ap()
    bi_ap = nc.dram_tensor("bi", [D], F32, kind="ExternalInput").ap()
    y_ap = nc.dram_tensor("y", [T, D], F32, kind="ExternalOutput").ap()

    with tile.TileContext(nc) as tc, ExitStack() as ctx:
        singles = ctx.enter_context(tc.tile_pool(name="singles", bufs=1))
        id_bf = singles.tile([P, P], BF16, tag="id_bf")
        id_f32 = singles.tile([P, P], F32, tag="id_f32")
        masks.make_identity(nc, id_bf[:])
        masks.make_identity(nc, id_f32[:])

        # biases as [e-within-block, eb] fp32 (per-partition bias scalars)
        bd_sb = singles.tile([P, EB], F32, tag="bd")
        nc.sync.dma_start(bd_sb[:], bd_ap.rearrange("(f p) -> p f", p=P))
        bi_sb = singles.tile([P, EB], F32, tag="bi")
        nc.sync.dma_start(bi_sb[:], bi_ap.rearrange("(f p) -> p f", p=P))

        # ---- weights: load (cast bf16) + PE-transpose to [d, e] layout ----
        wt_pool = ctx.enter_context(tc.tile_pool(name="wt", bufs=1))
        wstage = ctx.enter_context(tc.tile_pool(name="wstage", bufs=1))
        # PSUM pools (8 banks total):
        #   trx: weight/x bf16 transposes -> 2 banks
        #   try: y fp32 transposes        -> 2 banks
        #   pzd/pzi: matmul accumulators  -> 4 banks
        psum_trx = ctx.enter_context(tc.tile_pool(name="psum_trx", bufs=2, space="PSUM"))
        psum_try = ctx.enter_context(tc.tile_pool(name="psum_try", bufs=2, space="PSUM"))
        psum_mm = ctx.enter_context(tc.tile_pool(name="psum_mm", bufs=2, space="PSUM"))

        wT = {}
        for wi_idx, w_ap in enumerate((wd_ap, wi_ap)):
            wn = []
            for eb in range(EB):
                t_ = wstage.tile([P, D], BF16, tag=f"wn{eb}")
                nc.gpsimd.dma_start(t_[:], w_ap[eb * P:(eb + 1) * P, :])
                wn.append(t_)
            for db in range(DB):
                pw = psum_trx.tile([P, D], BF16, tag="trx")
                for eb in range(EB):
                    nc.tensor.transpose(
                        pw[:, eb * P:(eb + 1) * P],
                        wn[eb][:, db * P:(db + 1) * P],
                        id_bf[:],
                    )
                wt_tile = wt_pool.tile([P, D], BF16, tag=f"w{wi_idx}T{db}")
                nc.vector.tensor_copy(wt_tile[:], pw[:])
                wT[(wi_idx, db)] = wt_tile

        # ---- panel pipeline ----
        if XT_MODE == "dma":
            dram_pool = ctx.enter_context(
                tc.tile_pool(name="dram", bufs=1, space="DRAM"))
            xbf_dram = dram_pool.tile([T, D], BF16, tag="xbf_dram")
        xbf_pool = ctx.enter_context(tc.tile_pool(name="xbf", bufs=CFG["xbf"]))
        xt_pool = ctx.enter_context(tc.tile_pool(name="xt", bufs=CFG["xt"]))
        dec_pool = ctx.enter_context(tc.tile_pool(name="dec", bufs=CFG["dec"]))
        st_pool = ctx.enter_context(tc.tile_pool(name="st", bufs=CFG["st"]))
        y_pool = ctx.enter_context(tc.tile_pool(name="ysb", bufs=CFG["ysb"]))

        def load_xbf(p):
            """Issue the 4 cast-DMAs for panel p."""
            xbf = []
            for tk in range(TK):
                row0 = (p * TK + tk) * P
                t_ = xbf_pool.tile([P, D], BF16, tag=f"xbf{tk}")
                nc.gpsimd.dma_start(t_[:], x_ap[row0:row0 + P, :])
                xbf.append(t_)
            return xbf

        def transpose_db(xbf, db):
            """PE-transpose one d-block of a loaded panel -> xt tile."""
            pxt = psum_trx.tile([P, PANEL], BF16, tag="trx")
            for tk in range(TK):
                nc.tensor.transpose(
                    pxt[:, tk * P:(tk + 1) * P],
                    xbf[tk][:, db * P:(db + 1) * P],
                    id_bf[:],
                )
            xt_tile = xt_pool.tile([P, PANEL], BF16, tag=f"xt{db}")
            nc.vector.tensor_copy(xt_tile[:], pxt[:])
            return xt_tile

        prev_st = [None] * EB
        total = repeat * N_PANELS
        # prologue: panel 0 fully loaded + transposed
        xbf_next = load_xbf(0)
        xt = [transpose_db(xbf_next, db) for db in range(DB)]
        for p_rep in range(total):
            p = p_rep % N_PANELS
            if p_rep + 1 < total:
                xbf_next = load_xbf((p_rep + 1) % N_PANELS)
            xt_next = []

            # projections + scan, per output-channel block; next panel's
            # x-transposes interleave between the later MM groups so the
            # PE never idles long enough to re-throttle
            for eb in range(EB):
                pzd = psum_mm.tile([P, PANEL], F32, tag="pzd")
                for db in range(DB):
                    nc.tensor.matmul(
                        pzd[:],
                        wT[(0, db)][:, eb * P:(eb + 1) * P],
                        xt[db][:],
                        start=(db == 0),
                        stop=(db == DB - 1),
                    )
                pzi = psum_mm.tile([P, PANEL], F32, tag="pzi")
                for db in range(DB):
                    nc.tensor.matmul(
                        pzi[:],
                        wT[(1, db)][:, eb * P:(eb + 1) * P],
                        xt[db][:],
                        start=(db == 0),
                        stop=(db == DB - 1),
                    )

                if "act" in ablate:
                    continue
                dec = dec_pool.tile([P, PANEL], F32, tag="dec")
                nc.scalar.activation(
                    dec[:],
                    pzd[:],
                    mybir.ActivationFunctionType.Sigmoid,
                    bias=bd_sb[:, eb:eb + 1],
                    scale=1.0,
                )
                inj = dec_pool.tile([P, PANEL], F32, tag="inj")
                nc.scalar.activation(
                    inj[:],
                    pzi[:],
                    mybir.ActivationFunctionType.Identity,
                    bias=bi_sb[:, eb:eb + 1],
                    scale=1.0,
                )

                if "scan" in ablate:
                    continue
                st = st_pool.tile([P, PANEL], F32, tag=f"st{eb}")
                init = 0.0 if p_rep == 0 else prev_st[eb][:, PANEL - 1:PANEL]
                nc.vector.tensor_tensor_scan(
                    st[:],
                    dec[:],
                    inj[:],
                    init,
                    mybir.AluOpType.mult,
                    mybir.AluOpType.add,
                )
                prev_st[eb] = st

                if p_rep + 1 < total and eb >= EB - 4:
                    # 2 d-blocks of next panel's transposes per late eb group
                    for j in range(2):
                        db_n = (eb - (EB - 4)) * 2 + j
                        xt_next.append(transpose_db(xbf_next, db_n))

            if p_rep + 1 < total:
                xt = xt_next

            # transpose states back [e, t] -> [t, e] and store
            for tk in range(TK if ("ytr" not in ablate and "scan" not in ablate
                                   and "act" not in ablate) else 0):
                ysb = y_pool.tile([P, D], F32, tag="ysb")
                for half in range(2):
                    pyt = psum_try.tile([P, PANEL], F32, tag="try")
                    for j in range(4):
                        eb = half * 4 + j
                        nc.tensor.transpose(
                            pyt[:, j * P:(j + 1) * P],
                            prev_st[eb][:, tk * P:(tk + 1) * P],
                            id_f32[:],
                        )
                    nc.scalar.copy(ysb[:, half * PANEL:(half + 1) * PANEL], pyt[:])
                row0 = (p * TK + tk) * P
                nc.sync.dma_start(y_ap[row0:row0 + P, :], ysb[:])

    nc.compile()
    _cached_nc[key] = nc
    return nc


def run(inputs: dict, trace: bool = False):
    """Run on 8 cores; returns (output [8, T, D], BassKernelResults)."""
    nc = _build()
    x = np.asarray(inputs["x_seq"], dtype=np.float32)
    wd = np.ascontiguousarray(np.asarray(inputs["W_decay"], dtype=np.float32))
    bd = np.ascontiguousarray(np.asarray(inputs["b_decay"], dtype=np.float32))
    wi = np.ascontiguousarray(np.asarray(inputs["W_input"], dtype=np.float32))
    bi = np.ascontiguousarray(np.asarray(inputs["b_input"], dtype=np.float32))
    in_maps = [
        {
            "x": np.ascontiguousarray(x[b]),
            "wd": wd,
            "bd": bd,
            "wi": wi,
            "bi": bi,
        }
        for b in range(N_CORES)
    ]
    res = run_bass_kernel_spmd(
        nc, in_maps, core_ids=list(range(N_CORES)), trace=trace
    )
    out = np.stack([res.results[b]["y"] for b in range(N_CORES)], axis=0)
    return out, res


def kernel(x_seq, W_decay, b_decay, W_input, b_input) -> np.ndarray:
    out, _ = run(
        {
            "x_seq": x_seq,
            "W_decay": W_decay,
            "b_decay": b_decay,
            "W_input": W_input,
            "b_input": b_input,
        }
    )
    return out
